# revision 38
# baseline (speedup 1.0000x reference)
"""DETR-style matcher cost matrix on 8 Trainium2 NeuronCores.

cost[b, g, p] = -pred_cls[b, p, g]
                + mean(|pred_box[p] - gt_box[g]|)          (L1, 4 coords)
                + 1 - IoU + (area_c - union)/(area_c+eps)  (GIoU loss)
masked by gt_validity[b, g].

Sharding: data-parallel over batch, 4 batches per core (B=32, 8 cores).
Layout per (batch, gt-tile of 128): [128 part = gt rows, 900 free = preds].

All-fp16 pipeline sized for the DVE 2x/4x perf modes.  Coordinates are
host-shifted by -0.5 before the C=16 scale (halves the fp16 coordinate ulp).
Per axis (C-scaled):
  wi0' = min(P2,G2) - (max(P1,G1) + wg) = C*(wi0 - wg)
  wiR  = max(wi0', -wg) + wg            = C*relu(wi0)
  wc   = C*wp - wi0'                    = C*(enclosing width)
  inter = wiRx*wiRy ; areac = wcx*wcy ; union = (C2*ap + C2*ag) - inter
  cost = V*(0.5/C*(wcx+wcy) - 0.25*sp - 0.25*sg + 2
            - inter/(union+e) - union/(areac+e) - clsT)
Engine split: coordinate min/max TS + pair-fused TTs on DVE; areac/union
products on GPSIMD; hiR + exp(-ln(x+eps)) reciprocals + PSUM drain on the
scalar engine; the linear combine ((1/32)(wcx+wcy) - u1 - t2m - cls2) on the
PE via diag-stationary matmuls into PSUM, with validity V and the
V*(2-0.25*sg) bias applied by the drain.  pred_cls arrives host-transposed
fp16 with 0.25*sp pre-added (cls2 = clsT + 0.25*sp[p]); output is written
fp16 and upcast on the host.
"""

import numpy as np

B, Q = 32, 900
N_CORES = 8
B_PER = B // N_CORES
EPS = 1e-7
C = 16.0
C2 = C * C
SH = 0.5
GN_MAX = 2
GROUPS = [(0, 2), (2, 2), (4, 2), (6, 1)]  # (t0, gn) per batch, 7 full tiles

_cached = {}


def _split_multi_waits(nc):
    """This neuronxcc build rejects >1 sync-wait per instruction. Split any
    instruction carrying N>1 waits by inserting N-1 wait-carrier nops before
    it on the same (in-order) engine stream."""
    import concourse.mybir as mybir

    for fn in nc.m.functions:
        for bb in fn.blocks:
            out = []
            for ins in bb.instructions:
                si = getattr(ins, "sync_info", None)
                waits = list(si.on_wait) if (si and si.on_wait) else []
                if len(waits) > 1:
                    si.on_wait = [waits[-1]]
                    for j, w in enumerate(waits[:-1]):
                        nop = mybir.InstNoOp(name=f"{ins.name}-sw{j}", ins=[], outs=[])
                        nop.engine = ins.engine
                        nop.sync_info = mybir.SyncInfo(on_wait=[w], on_update=[])
                        out.append(nop)
                out.append(ins)
            bb.instructions[:] = out


def _build_nc():
    import concourse.bass as bass
    from concourse import mybir
    from concourse.tile import TileContext
    from concourse.masks import make_identity
    from concourse.alu_op_type import AluOpType as Alu

    f32 = mybir.dt.float32
    f16 = mybir.dt.float16
    Act = mybir.ActivationFunctionType

    nc = bass.Bass()
    clsT_d = nc.dram_tensor("clsT", [B_PER, Q, Q], f16, kind="ExternalInput")
    # pm16 rows: [WPm, HPm, APm, P1x, P1y, P2x, P2y]
    pm16_d = nc.dram_tensor("pm16", [B_PER, 7 * Q], f16, kind="ExternalInput")
    # g32 columns: Gx1', Gy1', Gx2', Gy2', WGc, HGc, AGc, nWGc, nHGc, V, b2
    g32_d = nc.dram_tensor("g32", [B_PER, Q, 11], f32, kind="ExternalInput")
    cost_d = nc.dram_tensor("cost", [B_PER, Q, Q], f16, kind="ExternalOutput")

    GX1, GY1, GX2, GY2, WG, HG, AG, NWG, NHG, VV, B2 = range(11)
    SEGS = ((0, 512), (512, Q))

    with TileContext(nc) as tc:
        with (
            tc.tile_pool(name="const", bufs=1) as constp,
            tc.tile_pool(name="batch", bufs=2) as batchp,
            tc.tile_pool(name="grp", bufs=2) as grp,
            tc.tile_pool(name="lnp", bufs=1) as lnp,
            tc.tile_pool(name="cls", bufs=2) as clsp,
            tc.tile_pool(name="outp", bufs=4) as outp,
            tc.tile_pool(name="rem", bufs=1) as remp,
            tc.tile_pool(name="psum", bufs=4, space="PSUM") as psp,
        ):
            identf = constp.tile([128, 128], f32)
            make_identity(nc, identf)
            stat32 = constp.tile([128, 128], f16)  # diag(1/32) = diag(0.5/C)
            nc.vector.tensor_scalar_mul(stat32[:], identf[:], 1.0 / 32.0)
            nident = constp.tile([128, 128], f16)  # diag(-1)
            nc.vector.tensor_scalar_mul(nident[:], identf[:], -1.0)
            epsb = constp.tile([128, 1], f32)  # eps bias for the Ln ops
            nc.gpsimd.memset(epsb[:], C2 * EPS)

            def emit_front(mapd, m16, t0, gn, s32, cls_src, relu_dve=False, apg_act=False):
                """Group front: everything up to the reciprocals."""
                AXY = grp.tile([128, 2, GN_MAX, Q], f16, tag="AXY")
                BXY = grp.tile([128, 2, GN_MAX, Q], f16, tag="BXY")
                WIHI = grp.tile([128, 2, GN_MAX, Q], f16, tag="WIHI")
                WIRH = grp.tile([128, 2, GN_MAX, Q], f16, tag="WIRH")
                WCHC = grp.tile([128, 2, GN_MAX, Q], f16, tag="WCHC")
                X = grp.tile([128, 3, GN_MAX, Q], f16, tag="X")  # inter|union|areac
                APG = grp.tile([128, GN_MAX, Q], f16, tag="APG")
                RCP = grp.tile([128, 2, GN_MAX, Q], f16, tag="RCP")
                clsm = clsp.tile([128, GN_MAX, Q], f16, tag="clsm")


                for i in range(gn):
                    t = t0 + i
                    src = cls_src(i)
                    if isinstance(src, list):
                        for (p0, p1), sap in src:
                            nc.sync.dma_start(out=clsm[p0:p1, i, :], in_=sap)
                    else:
                        nc.sync.dma_start(out=clsm[:, i, :], in_=src)
                    # coordinate min/max TS ops (fp16)
                    nc.vector.tensor_scalar(
                        AXY[:, 0, i, :], m16(5), s32(t, GX2), None, Alu.min
                    )
                    nc.vector.tensor_scalar(
                        AXY[:, 1, i, :], m16(6), s32(t, GY2), None, Alu.min
                    )
                    nc.vector.tensor_scalar(
                        BXY[:, 0, i, :], m16(3), s32(t, GX1), s32(t, WG),
                        Alu.max, Alu.add,
                    )
                    nc.vector.tensor_scalar(
                        BXY[:, 1, i, :], m16(4), s32(t, GY1), s32(t, HG),
                        Alu.max, Alu.add,
                    )
                    # APg = C2*ap + C2*ag
                    if apg_act:
                        nc.scalar.activation(
                            APG[:, i, :], m16(2), Act.Identity, bias=s32(t, AG)
                        )
                    else:
                        nc.vector.tensor_scalar(
                            APG[:, i, :], m16(2), s32(t, AG), None, Alu.add
                        )

                g = lambda tile: tile[:, :, 0:gn, :]
                # wi0' = [ax|ay] - [bx'|by']
                nc.vector.tensor_sub(g(WIHI), g(AXY), g(BXY))
                # wc = [WPm|HPm] - wi0'
                nc.vector.tensor_sub(g(WCHC), mapd[:, :, 0:gn, :], g(WIHI))
                for i in range(gn):
                    t = t0 + i
                    if relu_dve:
                        # relu = max(wi0', -wg) + wg   (DVE fast (MAX,ADD))
                        nc.vector.tensor_scalar(
                            WIRH[:, 0, i, :], WIHI[:, 0, i, :], s32(t, NWG),
                            s32(t, WG), Alu.max, Alu.add,
                        )
                        nc.vector.tensor_scalar(
                            WIRH[:, 1, i, :], WIHI[:, 1, i, :], s32(t, NHG),
                            s32(t, HG), Alu.max, Alu.add,
                        )
                    else:
                        nc.scalar.activation(
                            WIRH[:, 0, i, :], WIHI[:, 0, i, :], Act.Relu,
                            bias=s32(t, WG),
                        )
                        nc.scalar.activation(
                            WIRH[:, 1, i, :], WIHI[:, 1, i, :], Act.Relu,
                            bias=s32(t, HG),
                        )
                # inter = wiRx * wiRy
                nc.vector.tensor_mul(
                    X[:, 0, 0:gn, :], WIRH[:, 0, 0:gn, :], WIRH[:, 1, 0:gn, :]
                )
                # areac = wcx * wcy
                nc.vector.tensor_mul(
                    X[:, 2, 0:gn, :], WCHC[:, 0, 0:gn, :], WCHC[:, 1, 0:gn, :]
                )
                # union = APg - inter
                nc.vector.tensor_sub(
                    X[:, 1, 0:gn, :], APG[:, 0:gn, :], X[:, 0, 0:gn, :]
                )
                # rcu|rca = exp(-ln([union|areac] + eps))   (scalar engine)
                LN = lnp.tile([128, 2, GN_MAX, Q], f32, tag="LN")
                nc.scalar.activation(
                    LN[:, :, 0:gn, :], X[:, 1:3, 0:gn, :], Act.Ln, bias=epsb[:]
                )
                nc.scalar.activation(
                    RCP[:, :, 0:gn, :], LN[:, :, 0:gn, :], Act.Exp, scale=-1.0
                )
                return dict(WCHC=WCHC, X=X, RCP=RCP, clsm=clsm)

            def emit_back(ctx, t0, gn, s32, out_dst, drain_dve=False):
                """Group back: UT product, PE combine, drain, output DMA."""
                WCHC, X, RCP, clsm = ctx["WCHC"], ctx["X"], ctx["RCP"], ctx["clsm"]
                UT = grp.tile([128, 2, GN_MAX, Q], f16, tag="UT")
                # [u1|t2m] = [inter|union] * [rcu|rca]
                nc.vector.tensor_mul(
                    UT[:, :, 0:gn, :], X[:, 0:2, 0:gn, :], RCP[:, :, 0:gn, :]
                )
                for i in range(gn):
                    t = t0 + i
                    psum = psp.tile([128, 1024], f32, tag="ps")
                    for n0, n1 in SEGS:
                        nc.tensor.matmul(
                            psum[:, n0:n1], stat32[:], WCHC[:, 0, i, n0:n1],
                            start=True, stop=False,
                        )
                        nc.tensor.matmul(
                            psum[:, n0:n1], stat32[:], WCHC[:, 1, i, n0:n1],
                            start=False, stop=False,
                        )
                        nc.tensor.matmul(
                            psum[:, n0:n1], nident[:], UT[:, 0, i, n0:n1],
                            start=False, stop=False,
                        )
                        nc.tensor.matmul(
                            psum[:, n0:n1], nident[:], UT[:, 1, i, n0:n1],
                            start=False, stop=False,
                        )
                        nc.tensor.matmul(
                            psum[:, n0:n1], nident[:], clsm[:, i, n0:n1],
                            start=False, stop=True,
                        )
                    out16 = outp.tile([128, Q], f16, tag="out16")
                    if drain_dve:
                        # out = (psum * V) + b2 on the DVE (tail relief)
                        nc.vector.tensor_scalar(
                            out16[:], psum[:, 0:Q], s32(t, VV), s32(t, B2),
                            Alu.mult, Alu.add,
                        )
                    else:
                        nc.scalar.activation(
                            out16[:], psum[:, 0:Q], Act.Identity,
                            bias=s32(t, B2), scale=s32(t, VV),
                        )
                    dst = out_dst(i)
                    if isinstance(dst, list):
                        for (p0, p1), dd in dst:
                            nc.sync.dma_start(out=dd, in_=out16[p0:p1, :])
                    else:
                        nc.sync.dma_start(out=dst, in_=out16[:])

            # ================= main: 4 batches x 7 full gt tiles ============
            # Software pipeline: emit group g's back-half after group g+1's
            # front so no engine stream blocks on the cross-engine recip
            # chain.
            work = []  # (mapd, m16, t0, gn, s32, cls_src, out_dst)
            batch_tiles = []
            for b in range(B_PER):
                g32t = batchp.tile([128, 7, 11], f32, tag="g32")
                nc.sync.dma_start(
                    out=g32t[:],
                    in_=g32_d[b, 0:896, :].rearrange("(t p) s -> p t s", p=128),
                )
                pm16t = batchp.tile([128, 7 * Q], f16, tag="pm16")
                src16 = pm16_d[b][:].flatten()
                for c in (5, 6, 3, 4, 2, 0, 1):
                    bcast = bass.AP(
                        tensor=src16.tensor,
                        offset=src16.offset + Q * c,
                        ap=[[0, 128], [1, Q]],
                    )
                    nc.sync.dma_start(out=pm16t[:, Q * c : Q * (c + 1)], in_=bcast)
                mapd = batchp.tile([128, 2, GN_MAX, Q], f16, tag="mapd")
                for cc in range(2):
                    for slot in range(GN_MAX):
                        rep = bass.AP(
                            tensor=src16.tensor,
                            offset=src16.offset + Q * cc,
                            ap=[[0, 128], [1, Q]],
                        )
                        nc.sync.dma_start(out=mapd[:, cc, slot, :], in_=rep)
                s32 = lambda t, idx, g32t=g32t: g32t[:, t, idx : idx + 1]
                for t0, gn in GROUPS:
                    def cls_src(i, b=b, t0=t0):
                        gg = (t0 + i) * 128
                        return clsT_d[b, gg : gg + 128, :]

                    def out_dst(i, b=b, t0=t0):
                        gg = (t0 + i) * 128
                        return cost_d[b, gg : gg + 128, :]

                    m16 = lambda c, pm16t=pm16t: pm16t[:, c * Q : (c + 1) * Q]
                    work.append((mapd, m16, t0, gn, s32, cls_src, out_dst))
                if b == 0:
                    # emit batch-0 map DMAs first, then start the pipeline
                    # (remaining batches' DMAs flow in as groups are emitted)
                    pass

            # ---- packed remainder: rows 896:900 x 4 batches ----
            pm16R = remp.tile([128, 5 * Q], f16, tag="pm16R")  # rows 2..6
            nc.gpsimd.memset(pm16R[:], 1.0)
            mapdR = remp.tile([128, 2, 1, Q], f16, tag="mapdR")
            nc.gpsimd.memset(mapdR[:], 1.0)
            g32R = remp.tile([128, 1, 11], f32, tag="g32R")
            nc.gpsimd.memset(g32R[:], 0.0)
            for b in range(B_PER):
                src16 = pm16_d[b][:].flatten()
                bcast4 = bass.AP(
                    tensor=src16.tensor, offset=src16.offset + 2 * Q,
                    ap=[[0, 4], [1, 5 * Q]],
                )
                nc.sync.dma_start(out=pm16R[4 * b : 4 * b + 4, :], in_=bcast4)
                for cc in range(2):
                    rep4 = bass.AP(
                        tensor=src16.tensor, offset=src16.offset + Q * cc,
                        ap=[[0, 4], [1, Q]],
                    )
                    nc.sync.dma_start(
                        out=mapdR[4 * b : 4 * b + 4, cc, 0, :], in_=rep4
                    )
                nc.sync.dma_start(
                    out=g32R[4 * b : 4 * b + 4, 0, :], in_=g32_d[b, 896:900, :]
                )
            s32R = lambda t, idx: g32R[:, 0, idx : idx + 1]

            def cls_srcR(i):
                return [
                    ((4 * b, 4 * b + 4), clsT_d[b, 896:900, :])
                    for b in range(B_PER)
                ]

            def out_dstR(i):
                return [
                    ((4 * b, 4 * b + 4), cost_d[b, 896:900, :])
                    for b in range(B_PER)
                ]

            m16R = lambda c: pm16R[:, (c - 2) * Q : (c - 1) * Q]
            work.insert(-1, (mapdR, m16R, 0, 1, s32R, cls_srcR, out_dstR))

            # ---- pipelined emission ----
            # warmup: APg to the scalar engine (it idles until the first
            # wi0 products land); tail: relus and drains to the DVE (it
            # idles while the scalar engine drains its reciprocal queue).
            pending = None  # (ctx, t0, gn, s32, out_dst, drain_dve)
            nw = len(work)
            for wi, (mapd, pm16t, t0, gn, s32, cls_src, out_dst) in enumerate(work):
                ctx = emit_front(
                    mapd, pm16t, t0, gn, s32, cls_src,
                    relu_dve=wi >= nw - 4, apg_act=wi < 3,
                )
                if pending is not None:
                    emit_back(*pending)
                pending = (ctx, t0, gn, s32, out_dst, wi >= nw - 6)
            emit_back(*pending)
    _split_multi_waits(nc)
    return nc


def _get_nc():
    if "nc" not in _cached:
        _cached["nc"] = _build_nc()
    return _cached["nc"]


def _host_prep(pred_boxes, pred_cls, gt_boxes, gt_validity):
    """dtype/layout prep; per-element cost math stays on device."""
    pb = pred_boxes.astype(np.float32)
    gb = gt_boxes.astype(np.float32)
    V = gt_validity.astype(np.float32)
    Cf = np.float32(C)

    px1, py1, px2, py2 = pb[..., 0], pb[..., 1], pb[..., 2], pb[..., 3]
    gx1, gy1, gx2, gy2 = gb[..., 0], gb[..., 1], gb[..., 2], gb[..., 3]
    wp, hp = px2 - px1, py2 - py1
    wg, hg = gx2 - gx1, gy2 - gy1

    pm16 = np.stack(
        [
            Cf * wp, Cf * hp, np.float32(C2) * wp * hp,
            Cf * (px1 - SH), Cf * (py1 - SH), Cf * (px2 - SH), Cf * (py2 - SH),
        ],
        axis=1,
    )  # [B,7,Q]
    pm16 = np.ascontiguousarray(pm16.reshape(B, 7 * Q)).astype(np.float16)
    g32 = np.stack(
        [
            Cf * (gx1 - SH), Cf * (gy1 - SH), Cf * (gx2 - SH), Cf * (gy2 - SH),
            Cf * wg, Cf * hg, np.float32(C2) * wg * hg, -Cf * wg, -Cf * hg,
            V, V * (2.0 - 0.25 * (wg + hg)),
        ],
        axis=2,
    ).astype(np.float32)  # [B,Q,11]
    # cls2 = clsT + 0.25*sp[p]  (folds the 0.25*sp l1 term into the cls pass)
    sp4 = 0.25 * (wp + hp)
    cls2 = np.swapaxes(pred_cls, 1, 2) + sp4[:, None, :]
    cls2 = np.ascontiguousarray(cls2).astype(np.float16)

    maps = []
    for c in range(N_CORES):
        sl = slice(c * B_PER, (c + 1) * B_PER)
        maps.append(
            {
                "clsT": np.ascontiguousarray(cls2[sl]),
                "pm16": np.ascontiguousarray(pm16[sl]),
                "g32": np.ascontiguousarray(g32[sl]),
            }
        )
    return maps


def kernel(pred_boxes, pred_cls, gt_boxes, gt_validity, _trace=False):
    from concourse import bass_utils

    nc = _get_nc()
    maps = _host_prep(pred_boxes, pred_cls, gt_boxes, gt_validity)
    res = bass_utils.run_bass_kernel_spmd(
        nc, maps, core_ids=list(range(N_CORES)), trace=_trace
    )
    out = np.concatenate(
        [res.results[c]["cost"].astype(np.float32) for c in range(N_CORES)],
        axis=0,
    )
    if _trace:
        _cached["last_result"] = res
    return out


# revision 39
# speedup vs baseline: 1.0188x; 1.0188x over previous
"""DETR-style matcher cost matrix on 8 Trainium2 NeuronCores.

cost[b, g, p] = -pred_cls[b, p, g]
                + mean(|pred_box[p] - gt_box[g]|)          (L1, 4 coords)
                + 1 - IoU + (area_c - union)/(area_c+eps)  (GIoU loss)
masked by gt_validity[b, g].

Sharding: data-parallel over batch, 4 batches per core (B=32, 8 cores).
Layout per (batch, gt-tile of 128): [128 part = gt rows, 900 free = preds].

All-fp16 pipeline sized for the DVE 2x/4x perf modes.  Coordinates are
host-shifted by -0.5 before the C=16 scale (halves the fp16 coordinate ulp).
Per axis (C-scaled):
  wi0' = min(P2,G2) - (max(P1,G1) + wg) = C*(wi0 - wg)
  wiR  = max(wi0', -wg) + wg            = C*relu(wi0)
  wc   = C*wp - wi0'                    = C*(enclosing width)
  inter = wiRx*wiRy ; areac = wcx*wcy ; union = (C2*ap + C2*ag) - inter
  cost = V*(0.5/C*(wcx+wcy) - 0.25*sp - 0.25*sg + 2
            - inter/(union+e) - union/(areac+e) - clsT)
Engine split: coordinate min/max TS + pair-fused TTs on DVE; areac/union
products on GPSIMD; hiR + exp(-ln(x+eps)) reciprocals + PSUM drain on the
scalar engine; the linear combine ((1/32)(wcx+wcy) - u1 - t2m - cls2) on the
PE via diag-stationary matmuls into PSUM, with validity V and the
V*(2-0.25*sg) bias applied by the drain.  pred_cls arrives host-transposed
fp16 with 0.25*sp pre-added (cls2 = clsT + 0.25*sp[p]); output is written
fp16 and upcast on the host.
"""

import numpy as np

B, Q = 32, 900
N_CORES = 8
B_PER = B // N_CORES
EPS = 1e-7
C = 16.0
C2 = C * C
SH = 0.5
GN_MAX = 2
GROUPS = [(0, 2), (2, 2), (4, 2), (6, 1)]  # (t0, gn) per batch, 7 full tiles

_cached = {}


def _split_multi_waits(nc):
    """This neuronxcc build rejects >1 sync-wait per instruction. Split any
    instruction carrying N>1 waits by inserting N-1 wait-carrier nops before
    it on the same (in-order) engine stream."""
    import concourse.mybir as mybir

    for fn in nc.m.functions:
        for bb in fn.blocks:
            out = []
            for ins in bb.instructions:
                si = getattr(ins, "sync_info", None)
                waits = list(si.on_wait) if (si and si.on_wait) else []
                if len(waits) > 1:
                    si.on_wait = [waits[-1]]
                    for j, w in enumerate(waits[:-1]):
                        nop = mybir.InstNoOp(name=f"{ins.name}-sw{j}", ins=[], outs=[])
                        nop.engine = ins.engine
                        nop.sync_info = mybir.SyncInfo(on_wait=[w], on_update=[])
                        out.append(nop)
                out.append(ins)
            bb.instructions[:] = out


def _build_nc():
    import concourse.bass as bass
    from concourse import mybir
    from concourse.tile import TileContext
    from concourse.masks import make_identity
    from concourse.alu_op_type import AluOpType as Alu

    f32 = mybir.dt.float32
    f16 = mybir.dt.float16
    Act = mybir.ActivationFunctionType

    nc = bass.Bass()
    clsT_d = nc.dram_tensor("clsT", [B_PER, Q, Q], f16, kind="ExternalInput")
    # pm16 rows: [WPm, HPm, APm, P1x, P1y, P2x, P2y]
    pm16_d = nc.dram_tensor("pm16", [B_PER, 7 * Q], f16, kind="ExternalInput")
    # g32 columns: Gx1', Gy1', Gx2', Gy2', WGc, HGc, AGc, nWGc, nHGc, V, b2
    g32_d = nc.dram_tensor("g32", [B_PER, Q, 11], f32, kind="ExternalInput")
    cost_d = nc.dram_tensor("cost", [B_PER, Q, Q], f16, kind="ExternalOutput")

    GX1, GY1, GX2, GY2, WG, HG, AG, NWG, NHG, VV, B2 = range(11)
    SEGS = ((0, 512), (512, Q))

    with TileContext(nc) as tc:
        with (
            tc.tile_pool(name="const", bufs=1) as constp,
            tc.tile_pool(name="batch", bufs=2) as batchp,
            tc.tile_pool(name="grp", bufs=2) as grp,
            tc.tile_pool(name="lnp", bufs=1) as lnp,
            tc.tile_pool(name="cls", bufs=2) as clsp,
            tc.tile_pool(name="outp", bufs=4) as outp,
            tc.tile_pool(name="rem", bufs=1) as remp,
            tc.tile_pool(name="psum", bufs=4, space="PSUM") as psp,
        ):
            identf = constp.tile([128, 128], f32)
            make_identity(nc, identf)
            stat32 = constp.tile([128, 128], f16)  # diag(1/32) = diag(0.5/C)
            nc.vector.tensor_scalar_mul(stat32[:], identf[:], 1.0 / 32.0)
            nident = constp.tile([128, 128], f16)  # diag(-1)
            nc.vector.tensor_scalar_mul(nident[:], identf[:], -1.0)
            epsb = constp.tile([128, 1], f32)  # eps bias for the Ln ops
            nc.gpsimd.memset(epsb[:], C2 * EPS)

            def emit_front(mapd, m16, t0, gn, s32, cls_src, relu_dve=False, apg_act=False):
                """Group front: everything up to the reciprocals."""
                AXY = grp.tile([128, 2, GN_MAX, Q], f16, tag="AXY")
                BXY = grp.tile([128, 2, GN_MAX, Q], f16, tag="BXY")
                WIHI = grp.tile([128, 2, GN_MAX, Q], f16, tag="WIHI")
                WIRH = grp.tile([128, 2, GN_MAX, Q], f16, tag="WIRH")
                WCHC = grp.tile([128, 2, GN_MAX, Q], f16, tag="WCHC")
                X = grp.tile([128, 3, GN_MAX, Q], f16, tag="X")  # inter|union|areac
                APG = grp.tile([128, GN_MAX, Q], f16, tag="APG")
                RCP = grp.tile([128, 2, GN_MAX, Q], f16, tag="RCP")
                clsm = clsp.tile([128, GN_MAX, Q], f16, tag="clsm")


                for i in range(gn):
                    t = t0 + i
                    src = cls_src(i)
                    if isinstance(src, list):
                        for (p0, p1), sap in src:
                            nc.sync.dma_start(out=clsm[p0:p1, i, :], in_=sap)
                    else:
                        nc.sync.dma_start(out=clsm[:, i, :], in_=src)
                    # coordinate min/max TS ops (fp16)
                    nc.vector.tensor_scalar(
                        AXY[:, 0, i, :], m16(5), s32(t, GX2), None, Alu.min
                    )
                    nc.vector.tensor_scalar(
                        AXY[:, 1, i, :], m16(6), s32(t, GY2), None, Alu.min
                    )
                    nc.vector.tensor_scalar(
                        BXY[:, 0, i, :], m16(3), s32(t, GX1), s32(t, WG),
                        Alu.max, Alu.add,
                    )
                    nc.vector.tensor_scalar(
                        BXY[:, 1, i, :], m16(4), s32(t, GY1), s32(t, HG),
                        Alu.max, Alu.add,
                    )
                    # APg = C2*ap + C2*ag
                    if apg_act:
                        nc.scalar.activation(
                            APG[:, i, :], m16(2), Act.Identity, bias=s32(t, AG)
                        )
                    else:
                        nc.vector.tensor_scalar(
                            APG[:, i, :], m16(2), s32(t, AG), None, Alu.add
                        )

                g = lambda tile: tile[:, :, 0:gn, :]
                # wi0' = [ax|ay] - [bx'|by']
                nc.vector.tensor_sub(g(WIHI), g(AXY), g(BXY))
                # wc = [WPm|HPm] - wi0'
                nc.vector.tensor_sub(g(WCHC), mapd[:, :, 0:gn, :], g(WIHI))
                for i in range(gn):
                    t = t0 + i
                    if relu_dve:
                        # relu = max(wi0', -wg) + wg   (DVE fast (MAX,ADD))
                        nc.vector.tensor_scalar(
                            WIRH[:, 0, i, :], WIHI[:, 0, i, :], s32(t, NWG),
                            s32(t, WG), Alu.max, Alu.add,
                        )
                        nc.vector.tensor_scalar(
                            WIRH[:, 1, i, :], WIHI[:, 1, i, :], s32(t, NHG),
                            s32(t, HG), Alu.max, Alu.add,
                        )
                    else:
                        nc.scalar.activation(
                            WIRH[:, 0, i, :], WIHI[:, 0, i, :], Act.Relu,
                            bias=s32(t, WG),
                        )
                        nc.scalar.activation(
                            WIRH[:, 1, i, :], WIHI[:, 1, i, :], Act.Relu,
                            bias=s32(t, HG),
                        )
                # inter = wiRx * wiRy
                nc.vector.tensor_mul(
                    X[:, 0, 0:gn, :], WIRH[:, 0, 0:gn, :], WIRH[:, 1, 0:gn, :]
                )
                # areac = wcx * wcy
                nc.vector.tensor_mul(
                    X[:, 2, 0:gn, :], WCHC[:, 0, 0:gn, :], WCHC[:, 1, 0:gn, :]
                )
                # union = APg - inter
                nc.vector.tensor_sub(
                    X[:, 1, 0:gn, :], APG[:, 0:gn, :], X[:, 0, 0:gn, :]
                )
                # rcu|rca = exp(-ln([union|areac] + eps))   (scalar engine)
                LN = lnp.tile([128, 2, GN_MAX, Q], f32, tag="LN")
                nc.scalar.activation(
                    LN[:, :, 0:gn, :], X[:, 1:3, 0:gn, :], Act.Ln, bias=epsb[:]
                )
                nc.scalar.activation(
                    RCP[:, :, 0:gn, :], LN[:, :, 0:gn, :], Act.Exp, scale=-1.0
                )
                return dict(WCHC=WCHC, X=X, RCP=RCP, clsm=clsm)

            def emit_back(ctx, t0, gn, s32, out_dst, drain_dve=False):
                """Group back: UT product, PE combine, drain, output DMA."""
                WCHC, X, RCP, clsm = ctx["WCHC"], ctx["X"], ctx["RCP"], ctx["clsm"]
                UT = grp.tile([128, 2, GN_MAX, Q], f16, tag="UT")
                # [u1|t2m] = [inter|union] * [rcu|rca]
                nc.vector.tensor_mul(
                    UT[:, :, 0:gn, :], X[:, 0:2, 0:gn, :], RCP[:, :, 0:gn, :]
                )
                for i in range(gn):
                    t = t0 + i
                    psum = psp.tile([128, 1024], f32, tag="ps")
                    for n0, n1 in SEGS:
                        nc.tensor.matmul(
                            psum[:, n0:n1], stat32[:], WCHC[:, 0, i, n0:n1],
                            start=True, stop=False,
                        )
                        nc.tensor.matmul(
                            psum[:, n0:n1], stat32[:], WCHC[:, 1, i, n0:n1],
                            start=False, stop=False,
                        )
                        nc.tensor.matmul(
                            psum[:, n0:n1], nident[:], UT[:, 0, i, n0:n1],
                            start=False, stop=False,
                        )
                        nc.tensor.matmul(
                            psum[:, n0:n1], nident[:], UT[:, 1, i, n0:n1],
                            start=False, stop=False,
                        )
                        nc.tensor.matmul(
                            psum[:, n0:n1], nident[:], clsm[:, i, n0:n1],
                            start=False, stop=True,
                        )
                    out16 = outp.tile([128, Q], f16, tag="out16")
                    if drain_dve:
                        # out = (psum * V) + b2 on the DVE (tail relief)
                        nc.vector.tensor_scalar(
                            out16[:], psum[:, 0:Q], s32(t, VV), s32(t, B2),
                            Alu.mult, Alu.add,
                        )
                    else:
                        nc.scalar.activation(
                            out16[:], psum[:, 0:Q], Act.Identity,
                            bias=s32(t, B2), scale=s32(t, VV),
                        )
                    dst = out_dst(i)
                    if isinstance(dst, list):
                        for (p0, p1), dd in dst:
                            nc.sync.dma_start(out=dd, in_=out16[p0:p1, :])
                    else:
                        nc.sync.dma_start(out=dst, in_=out16[:])

            # ================= main: 4 batches x 7 full gt tiles ============
            # Software pipeline: emit group g's back-half after group g+1's
            # front so no engine stream blocks on the cross-engine recip
            # chain.
            work = []  # (mapd, m16, t0, gn, s32, cls_src, out_dst)
            batch_tiles = []
            for b in range(B_PER):
                g32t = batchp.tile([128, 7, 11], f32, tag="g32")
                nc.sync.dma_start(
                    out=g32t[:],
                    in_=g32_d[b, 0:896, :].rearrange("(t p) s -> p t s", p=128),
                )
                pm16t = batchp.tile([128, 7 * Q], f16, tag="pm16")
                src16 = pm16_d[b][:].flatten()
                for c in (5, 6, 3, 4, 2, 0, 1):
                    bcast = bass.AP(
                        tensor=src16.tensor,
                        offset=src16.offset + Q * c,
                        ap=[[0, 128], [1, Q]],
                    )
                    nc.sync.dma_start(out=pm16t[:, Q * c : Q * (c + 1)], in_=bcast)
                mapd = batchp.tile([128, 2, GN_MAX, Q], f16, tag="mapd")
                for cc in range(2):
                    for slot in range(GN_MAX):
                        rep = bass.AP(
                            tensor=src16.tensor,
                            offset=src16.offset + Q * cc,
                            ap=[[0, 128], [1, Q]],
                        )
                        nc.sync.dma_start(out=mapd[:, cc, slot, :], in_=rep)
                s32 = lambda t, idx, g32t=g32t: g32t[:, t, idx : idx + 1]
                for t0, gn in GROUPS:
                    def cls_src(i, b=b, t0=t0):
                        gg = (t0 + i) * 128
                        return clsT_d[b, gg : gg + 128, :]

                    def out_dst(i, b=b, t0=t0):
                        gg = (t0 + i) * 128
                        return cost_d[b, gg : gg + 128, :]

                    m16 = lambda c, pm16t=pm16t: pm16t[:, c * Q : (c + 1) * Q]
                    work.append((mapd, m16, t0, gn, s32, cls_src, out_dst))
                if b == 0:
                    # emit batch-0 map DMAs first, then start the pipeline
                    # (remaining batches' DMAs flow in as groups are emitted)
                    pass

            # ---- packed remainder: rows 896:900 x 4 batches ----
            pm16R = remp.tile([128, 5 * Q], f16, tag="pm16R")  # rows 2..6
            nc.gpsimd.memset(pm16R[:], 1.0)
            mapdR = remp.tile([128, 2, 1, Q], f16, tag="mapdR")
            nc.gpsimd.memset(mapdR[:], 1.0)
            g32R = remp.tile([128, 1, 11], f32, tag="g32R")
            nc.gpsimd.memset(g32R[:], 0.0)
            for b in range(B_PER):
                src16 = pm16_d[b][:].flatten()
                bcast4 = bass.AP(
                    tensor=src16.tensor, offset=src16.offset + 2 * Q,
                    ap=[[0, 4], [1, 5 * Q]],
                )
                nc.sync.dma_start(out=pm16R[4 * b : 4 * b + 4, :], in_=bcast4)
                for cc in range(2):
                    rep4 = bass.AP(
                        tensor=src16.tensor, offset=src16.offset + Q * cc,
                        ap=[[0, 4], [1, Q]],
                    )
                    nc.sync.dma_start(
                        out=mapdR[4 * b : 4 * b + 4, cc, 0, :], in_=rep4
                    )
                nc.sync.dma_start(
                    out=g32R[4 * b : 4 * b + 4, 0, :], in_=g32_d[b, 896:900, :]
                )
            s32R = lambda t, idx: g32R[:, 0, idx : idx + 1]

            def cls_srcR(i):
                return [
                    ((4 * b, 4 * b + 4), clsT_d[b, 896:900, :])
                    for b in range(B_PER)
                ]

            def out_dstR(i):
                return [
                    ((4 * b, 4 * b + 4), cost_d[b, 896:900, :])
                    for b in range(B_PER)
                ]

            m16R = lambda c: pm16R[:, (c - 2) * Q : (c - 1) * Q]
            work.insert(-1, (mapdR, m16R, 0, 1, s32R, cls_srcR, out_dstR))

            # ---- pipelined emission ----
            # warmup: APg to the scalar engine (it idles until the first
            # wi0 products land); tail: relus and drains to the DVE (it
            # idles while the scalar engine drains its reciprocal queue).
            pending = None  # (ctx, t0, gn, s32, out_dst, drain_dve)
            nw = len(work)
            for wi, (mapd, pm16t, t0, gn, s32, cls_src, out_dst) in enumerate(work):
                ctx = emit_front(
                    mapd, pm16t, t0, gn, s32, cls_src,
                    relu_dve=wi >= nw - 4, apg_act=wi < 3,
                )
                if pending is not None:
                    emit_back(*pending)
                pending = (ctx, t0, gn, s32, out_dst, wi >= nw - 5)
            emit_back(*pending)
    _split_multi_waits(nc)
    return nc


def _get_nc():
    if "nc" not in _cached:
        _cached["nc"] = _build_nc()
    return _cached["nc"]


def _host_prep(pred_boxes, pred_cls, gt_boxes, gt_validity):
    """dtype/layout prep; per-element cost math stays on device."""
    pb = pred_boxes.astype(np.float32)
    gb = gt_boxes.astype(np.float32)
    V = gt_validity.astype(np.float32)
    Cf = np.float32(C)

    px1, py1, px2, py2 = pb[..., 0], pb[..., 1], pb[..., 2], pb[..., 3]
    gx1, gy1, gx2, gy2 = gb[..., 0], gb[..., 1], gb[..., 2], gb[..., 3]
    wp, hp = px2 - px1, py2 - py1
    wg, hg = gx2 - gx1, gy2 - gy1

    pm16 = np.stack(
        [
            Cf * wp, Cf * hp, np.float32(C2) * wp * hp,
            Cf * (px1 - SH), Cf * (py1 - SH), Cf * (px2 - SH), Cf * (py2 - SH),
        ],
        axis=1,
    )  # [B,7,Q]
    pm16 = np.ascontiguousarray(pm16.reshape(B, 7 * Q)).astype(np.float16)
    g32 = np.stack(
        [
            Cf * (gx1 - SH), Cf * (gy1 - SH), Cf * (gx2 - SH), Cf * (gy2 - SH),
            Cf * wg, Cf * hg, np.float32(C2) * wg * hg, -Cf * wg, -Cf * hg,
            V, V * (2.0 - 0.25 * (wg + hg)),
        ],
        axis=2,
    ).astype(np.float32)  # [B,Q,11]
    # cls2 = clsT + 0.25*sp[p]  (folds the 0.25*sp l1 term into the cls pass)
    sp4 = 0.25 * (wp + hp)
    cls2 = np.swapaxes(pred_cls, 1, 2) + sp4[:, None, :]
    cls2 = np.ascontiguousarray(cls2).astype(np.float16)

    maps = []
    for c in range(N_CORES):
        sl = slice(c * B_PER, (c + 1) * B_PER)
        maps.append(
            {
                "clsT": np.ascontiguousarray(cls2[sl]),
                "pm16": np.ascontiguousarray(pm16[sl]),
                "g32": np.ascontiguousarray(g32[sl]),
            }
        )
    return maps


def kernel(pred_boxes, pred_cls, gt_boxes, gt_validity, _trace=False):
    from concourse import bass_utils

    nc = _get_nc()
    maps = _host_prep(pred_boxes, pred_cls, gt_boxes, gt_validity)
    res = bass_utils.run_bass_kernel_spmd(
        nc, maps, core_ids=list(range(N_CORES)), trace=_trace
    )
    out = np.concatenate(
        [res.results[c]["cost"].astype(np.float32) for c in range(N_CORES)],
        axis=0,
    )
    if _trace:
        _cached["last_result"] = res
    return out
